# revision 26
# baseline (speedup 1.0000x reference)
"""Trainium2 Bass kernel for nn_BinLoss_7103875908252.

Computes: labels = histogram-bin(target) -> combined bin id in [0, 1024);
          loss = mean_i ||features_i - centers[labels_i]||^2   (clip is a
          no-op for this data regime).

Sharding: data-parallel over the batch axis across 8 NeuronCores
(4096 rows each).  Each core returns a partial sum; host sums and
divides by N.

Per-core layout: row i = p*32 + r lives in partition p, slot r.
Strategy (v3: fp8 tables + batched ops + subtract folded into PE):
  - centers loaded once f32 (2MB), cast to fp8(e4m3) on ACT (loss bias
    of fp8 centers ~2e-4, tolerance 2e-2): cent8_sb feeds the PE path,
    table8 (DRAM) feeds the indirect-DMA gather path (512B rows).
  - binning: count of (v > edge_j) over 31 exact f32 edges on DVE in 2
    groups (PE slots 16..31 first), bit-exact vs searchsorted 'left'.
  - slots 0..15: SWDGE indirect gather from table8, 4 slots per call;
    d = F - g8 on DVE (4-slot batches), Square+accum on ACT.
  - slots 16..31: PE fp8 DoubleRow one-hot gather, 2-tile groups:
      sel2[k, h, c, i] = (label_i == 128c + k) as fp8, DVE is_equal of
      the PE-transposed labels (read straight from PSUM) vs fp16 iota;
      per tile: 4 DR MMs (256-bin contraction each) + one final
      negated-identity f32r matmul that subtracts F, so PSUM ends as
      (G - F) and ACT squares PSUM directly ([P,2,512] per group).
  - finish: reduce acc, 128->1 via ones matmul (PE), DMA out [1,1].
"""

import numpy as np

P = 128           # partitions
R = 32            # rows per partition per core
D = 512           # feature dim
K = 1024          # number of centers
NCORES = 8
N = 32768
SHARD = N // NCORES            # 4096
assert SHARD == P * R

NKC = K // P      # 8 center chunks of 128 rows
XG = 24           # slots gathered via indirect DMA (slots 0..XG-1)
YPE = R - XG      # slots via PE DoubleRow one-hot (slots XG..31)
NG = YPE // 2     # PE 2-tile groups

# f32 bit patterns of jnp.linspace(0.0, 1.0, 31, dtype=float32)
EDGE_BITS = [
    0x00000000, 0x3d088889, 0x3d888889, 0x3dccccce, 0x3e088889, 0x3e2aaaab,
    0x3e4cccce, 0x3e6eeef0, 0x3e888889, 0x3e99999a, 0x3eaaaaab, 0x3ebbbbbc,
    0x3eccccce, 0x3edddddf, 0x3eeeeef0, 0x3f000000, 0x3f088889, 0x3f111112,
    0x3f19999a, 0x3f222223, 0x3f2aaaab, 0x3f333334, 0x3f3bbbbc, 0x3f444445,
    0x3f4cccce, 0x3f555556, 0x3f5ddddf, 0x3f666667, 0x3f6eeef0, 0x3f777778,
    0x3f800000,
]
EDGES = [float(np.uint32(b).view(np.float32)) for b in EDGE_BITS]
NE = len(EDGES)   # 31

_CACHE = {}


def build_bass():
    """Build + compile the per-core Bass/Tile kernel (SPMD, same NEFF on
    all 8 cores)."""
    from contextlib import ExitStack

    import concourse.bacc as bacc
    import concourse.tile as tile
    from concourse import bass, mybir

    f32 = mybir.dt.float32
    f32r = mybir.dt.float32r
    fp16 = mybir.dt.float16
    bf16 = mybir.dt.bfloat16
    fp8 = mybir.dt.float8e4
    i32 = mybir.dt.int32
    DR = mybir.MatmulPerfMode.DoubleRow

    nc = bacc.Bacc(
        "TRN2", target_bir_lowering=False, debug=False, num_devices=NCORES
    )
    feat = nc.dram_tensor("features", [SHARD, D], f32, kind="ExternalInput").ap()
    targ = nc.dram_tensor("target", [SHARD, 2], f32, kind="ExternalInput").ap()
    cent = nc.dram_tensor("centers", [K, D], f32, kind="ExternalInput").ap()
    out = nc.dram_tensor("out", [1, 1], f32, kind="ExternalOutput").ap()

    with tile.TileContext(nc) as tc, ExitStack() as ctx:
        const_p = ctx.enter_context(tc.tile_pool(name="const", bufs=1))
        work_p = ctx.enter_context(tc.tile_pool(name="work", bufs=1))
        gat_p = ctx.enter_context(tc.tile_pool(name="gat", bufs=6))
        sel_p = ctx.enter_context(tc.tile_pool(name="sel", bufs=3))
        dif_p = ctx.enter_context(tc.tile_pool(name="dif", bufs=3))
        psum_t = ctx.enter_context(tc.tile_pool(name="psumt", bufs=2, space="PSUM"))
        psum_d = ctx.enter_context(tc.tile_pool(name="psumd", bufs=2, space="PSUM"))
        dram_p = ctx.enter_context(tc.tile_pool(name="dram", bufs=1, space="DRAM"))

        # ---- sync-queue DMAs: target, centers, then features (ordered)
        ttile = work_p.tile([P, R, 2], f32)
        nc.sync.dma_start(ttile[:], targ.rearrange("(p r) c -> p r c", p=P))

        F = work_p.tile([P, R, D], f32)
        feat_re = feat.rearrange("(p r) d -> p r d", p=P)
        # PE slots (16..31) early so the PE pipeline is fed; gather-slot
        # chunks interleave to match the SWDGE gather/subtract timeline.
        forder = [(24, 26), (26, 30), (0, 4), (30, 32), (4, 8), (8, 12),
                  (12, 16), (16, 20), (20, 24)]
        for (a, b) in forder:
            nc.sync.dma_start(F[:, a:b, :], feat_re[:, a:b, :])

        # ---- fp8 centers tiles (cast emitted in the master schedule)
        cent8_sb = work_p.tile([P, NKC, D], fp8)
        table8 = dram_p.tile([K, D], fp8)

        # ---- consts
        etile = const_p.tile([P, NE], f32)
        for j, e in enumerate(EDGES):
            nc.vector.memset(etile[:, j : j + 1], e)

        # identities: f32 for transposes, fp8 + negated-f32 for the
        # gather-path subtract-in-PSUM matmuls
        i8 = const_p.tile([P, P], fp8)
        nc.gpsimd.memset(i8[:], 0.0)
        nc.gpsimd.affine_select(
            out=i8[:],
            in_=i8[:],
            compare_op=mybir.AluOpType.not_equal,
            fill=1.0,
            base=0,
            pattern=[[-1, P]],
            channel_multiplier=1,
        )
        negI = const_p.tile([P, P], f32)
        nc.gpsimd.memset(negI[:], 0.0)
        nc.gpsimd.affine_select(
            out=negI[:],
            in_=negI[:],
            compare_op=mybir.AluOpType.not_equal,
            fill=-1.0,
            base=0,
            pattern=[[-1, P]],
            channel_multiplier=1,
        )
        identity = const_p.tile([P, P], f32)
        nc.gpsimd.memset(identity[:], 0.0)
        nc.gpsimd.affine_select(
            out=identity[:],
            in_=identity[:],
            compare_op=mybir.AluOpType.not_equal,
            fill=1.0,
            base=0,
            pattern=[[-1, P]],
            channel_multiplier=1,
        )
        # iota16[j, c] = 128*c + j (bin id of partition j in chunk c)
        iota16 = const_p.tile([P, NKC], fp16)
        nc.gpsimd.iota(
            iota16[:],
            pattern=[[P, NKC]],
            base=0,
            channel_multiplier=1,
            allow_small_or_imprecise_dtypes=True,
        )
        iota_full = const_p.tile([P, NKC, P], fp16)
        nc.vector.tensor_copy(
            out=iota_full[:], in_=iota16[:].unsqueeze(2).broadcast_to([P, NKC, P])
        )
        acc = work_p.tile([P, R], f32)
        nc.vector.memset(acc[:], 0.0)
        ones = const_p.tile([P, 1], f32)
        nc.vector.memset(ones[:], 1.0)

        # ---- binning (DVE), 2 groups: PE slots first
        labi = work_p.tile([P, R, 1], i32)
        labf2 = work_p.tile([P, R, 1], f32)
        bins = work_p.tile([P, R, 2], f32)
        labf = work_p.tile([P, R, 1], f32)
        cmp = work_p.tile([P, 2 * XG, NE], f32)

        def bin_group(h0, h1):
            rs = slice(h0, h1)
            w = 2 * (h1 - h0)
            tvals = ttile[:, rs, :].rearrange("p r c -> p (r c)")
            nc.vector.tensor_tensor(
                out=cmp[:, :w, :],
                in0=tvals.unsqueeze(2).broadcast_to([P, w, NE]),
                in1=etile[:].unsqueeze(1).broadcast_to([P, w, NE]),
                op=mybir.AluOpType.is_gt,
            )
            nc.vector.tensor_reduce(
                out=bins[:, rs, :].rearrange("p r c -> p (r c)"),
                in_=cmp[:, :w, :],
                axis=mybir.AxisListType.X,
                op=mybir.AluOpType.add,
            )
            nc.vector.tensor_scalar(
                out=labf[:, rs, :],
                in0=bins[:, rs, 0:1],
                scalar1=float(32.0),
                scalar2=None,
                op0=mybir.AluOpType.mult,
            )
            nc.vector.tensor_tensor(
                out=labf2[:, rs, :],
                in0=labf[:, rs, :],
                in1=bins[:, rs, 1:2],
                op=mybir.AluOpType.add,
            )
            nc.vector.tensor_copy(out=labi[:, rs, :], in_=labf2[:, rs, :])


        # ---- gather path (slots 0..XG-1), 4 slots per buffer; the
        # subtract runs on PE: psum_d2 = I8.T @ g8 + (-I).T @ F, so DVE
        # never touches gather slots and ACT squares PSUM directly.
        GGRP = [(4 * k, 4 * k + 4) for k in range(XG // 4)]
        g8t = {}

        def gen4(k):
            a, b = GGRP[k]
            g8 = gat_p.tile([P, 4, D], fp8, tag="g8")
            for t in range(b - a):
                nc.gpsimd.indirect_dma_start(
                    out=g8[:, t, :],
                    out_offset=None,
                    in_=table8[:, :],
                    in_offset=bass.IndirectOffsetOnAxis(
                        ap=labi[:, a + t, :], axis=0
                    ),
                )
            g8t[k] = g8

        def gsub(k):
            a, b = GGRP[k]
            g8 = g8t.pop(k)
            for half in range(2):
                gd = psum_d.tile([P, 2, D], f32, tag="gp2")
                # batch same-weight matmuls to avoid LDWEIGHTS thrash
                for t in range(2):
                    nc.tensor.matmul(
                        out=gd[:, t, :],
                        lhsT=i8[:],
                        rhs=g8[:, 2 * half + t, :],
                        start=True,
                        stop=False,
                        skip_group_check=True,
                    )
                for t in range(2):
                    nc.tensor.matmul(
                        out=gd[:, t, :],
                        lhsT=negI[:],
                        rhs=F[:, a + 2 * half + t, :],
                        start=False,
                        stop=True,
                        skip_group_check=True,
                    )
                nc.scalar.activation(
                    out=gd[:],
                    in_=gd[:],
                    func=mybir.ActivationFunctionType.Square,
                    accum_out=acc[:, a + 2 * half : a + 2 * half + 1],
                )

        # ---- PE DoubleRow one-hot path (slots XG..31), 2-tile groups
        psT2s = {}
        sels = {}
        gp2s = {}

        def TT(G):
            psT2 = psum_t.tile([P, 2, P], f32, tag="psT")
            for h in range(2):
                s = XG + 2 * G + h
                nc.tensor.transpose(
                    out=psT2[:, h, :],
                    in_=labf2[:, s, :].to_broadcast([P, P]),
                    identity=identity[:],
                )
            psT2s[G] = psT2

        def sel2(G):
            psT2 = psT2s.pop(G)
            sel = sel_p.tile([P, 2, NKC, P], fp8, tag="sel")
            nc.vector.tensor_tensor(
                out=sel[:],
                in0=psT2[:].unsqueeze(2).broadcast_to([P, 2, NKC, P]),
                in1=iota_full[:].unsqueeze(1).broadcast_to([P, 2, NKC, P]),
                op=mybir.AluOpType.is_equal,
            )
            sels[G] = sel

        def mm2(G):
            sel = sels.pop(G)
            gp2 = psum_d.tile([P, 2, D], f32, tag="gp2")
            for h in range(2):
                for c2 in range(NKC // 2):
                    nc.tensor.matmul(
                        out=gp2[:, h, :],
                        lhsT=sel[:, h, 2 * c2 : 2 * c2 + 2, :],
                        rhs=cent8_sb[:, 2 * c2 : 2 * c2 + 2, :],
                        start=(c2 == 0),
                        stop=(c2 == NKC // 2 - 1),
                        perf_mode=DR,
                    )
            gp2s[G] = gp2

        def sq2(G):
            gp2 = gp2s.pop(G)
            s0 = XG + 2 * G
            dt_ = dif_p.tile([P, 2, D], bf16, tag="d2")
            nc.vector.tensor_tensor(
                out=dt_[:],
                in0=F[:, s0 : s0 + 2, :],
                in1=gp2[:],
                op=mybir.AluOpType.subtract,
            )
            nc.scalar.activation(
                out=dt_[:],
                in_=dt_[:],
                func=mybir.ActivationFunctionType.Square,
                accum_out=acc[:, s0 : s0 + 1],
            )

        # ---- master emission schedule
        nc.gpsimd.dma_start(out=table8[:, :], in_=cent[:, :])
        nc.gpsimd.dma_start(    # before gens: table8 write->read dep
            cent8_sb[:, :, :], table8[:].rearrange("(c p) d -> p c d", p=P)
        )
        bin_group(0, XG)        # gather slots first
        for k in range(len(GGRP)):
            gen4(k)
        bin_group(XG, R)

        TT(0)
        sel2(0)
        for G in range(NG):
            if G + 1 < NG:
                TT(G + 1)
                sel2(G + 1)
            mm2(G)
            sq2(G)
        for k in range(len(GGRP)):
            gsub(k)

        # ---- final reduction
        s_ = work_p.tile([P, 1], f32)
        nc.vector.tensor_reduce(
            out=s_[:], in_=acc[:], axis=mybir.AxisListType.X, op=mybir.AluOpType.add
        )
        ps = psum_t.tile([1, 1], f32)
        nc.tensor.matmul(out=ps[:], lhsT=ones[:], rhs=s_[:], start=True, stop=True)
        res = work_p.tile([1, 1], f32)
        nc.vector.tensor_copy(out=res[:], in_=ps[:])
        nc.sync.dma_start(out[:, :], res[:])

    nc.compile()
    return nc


def _get_nc():
    if "nc" not in _CACHE:
        _CACHE["nc"] = build_bass()
    return _CACHE["nc"]


def kernel(features, target, centers):
    from concourse.bass_utils import run_bass_kernel_spmd

    features = np.ascontiguousarray(features, dtype=np.float32)
    target = np.ascontiguousarray(target, dtype=np.float32)
    centers = np.ascontiguousarray(centers, dtype=np.float32)

    nc = _get_nc()
    in_maps = []
    for c in range(NCORES):
        sl = slice(c * SHARD, (c + 1) * SHARD)
        in_maps.append(
            {
                "features": np.ascontiguousarray(features[sl]),
                "target": np.ascontiguousarray(target[sl]),
                "centers": centers,
            }
        )
    r = run_bass_kernel_spmd(
        nc,
        in_maps,
        core_ids=list(range(NCORES)),
        trace=_CACHE.get("trace", False),
        tmpdir=_CACHE.get("tmpdir"),
    )
    _CACHE["last_results"] = r
    total = sum(float(res["out"][0, 0]) for res in r.results)
    return np.float32(total / N)


# revision 27
# speedup vs baseline: 1.1241x; 1.1241x over previous
"""Trainium2 Bass kernel for nn_BinLoss_7103875908252.

Computes: labels = histogram-bin(target) -> combined bin id in [0, 1024);
          loss = mean_i ||features_i - centers[labels_i]||^2   (clip is a
          no-op for this data regime).

Sharding: data-parallel over the batch axis across 8 NeuronCores
(4096 rows each).  Each core returns a partial sum; host sums and
divides by N.

Per-core layout: row i = p*32 + r lives in partition p, slot r.
Strategy (final): fp8 center tables + split gather across SWDGE and PE:
  - centers f32 -> fp8(e4m3) via one SWDGE DRAM->DRAM cast (table8,
    feeds the indirect-DMA gather path with 512B rows) + a 0.5MB
    readback into SBUF (cent8_sb, feeds the PE one-hot path).  fp8
    centers bias the loss by ~2e-4; tolerance is 2e-2.
  - binning: count of (v > edge_j) over the 31 exact f32 edges on DVE
    (bit-exact vs jnp.searchsorted side='left'), gather slots first.
  - slots 0..XG-1 (24): SWDGE indirect gather from table8 (the serial
    GPSIMD descriptor generation, ~1.1us/128 rows, is the critical
    chain); d = F - g8 on DVE in 4-slot batches, Square+accum on ACT.
  - slots XG..31 (8): PE fp8 DoubleRow one-hot gather, 2-tile groups:
    sel2[k, h, c, i] = (label_i == 128c + k) as fp8 via DVE is_equal of
    PE-transposed labels (read from PSUM) vs an fp16 iota; 4 DR MMs
    per tile (256-bin contraction, N=512) accumulate G in PSUM; DVE
    subtracts F, ACT squares with row-accumulate.
  - feature chunks are DMAed PE-slots-first, then interleaved with
    gather-slot chunks to match the gather timeline.
  - finish: reduce acc, 128->1 via ones matmul (PE), DMA out [1,1].
"""

import numpy as np

P = 128           # partitions
R = 32            # rows per partition per core
D = 512           # feature dim
K = 1024          # number of centers
NCORES = 8
N = 32768
SHARD = N // NCORES            # 4096
assert SHARD == P * R

NKC = K // P      # 8 center chunks of 128 rows
XG = 24           # slots gathered via indirect DMA (slots 0..XG-1)
YPE = R - XG      # slots via PE DoubleRow one-hot (slots XG..31)
NG = YPE // 2     # PE 2-tile groups

# f32 bit patterns of jnp.linspace(0.0, 1.0, 31, dtype=float32)
EDGE_BITS = [
    0x00000000, 0x3d088889, 0x3d888889, 0x3dccccce, 0x3e088889, 0x3e2aaaab,
    0x3e4cccce, 0x3e6eeef0, 0x3e888889, 0x3e99999a, 0x3eaaaaab, 0x3ebbbbbc,
    0x3eccccce, 0x3edddddf, 0x3eeeeef0, 0x3f000000, 0x3f088889, 0x3f111112,
    0x3f19999a, 0x3f222223, 0x3f2aaaab, 0x3f333334, 0x3f3bbbbc, 0x3f444445,
    0x3f4cccce, 0x3f555556, 0x3f5ddddf, 0x3f666667, 0x3f6eeef0, 0x3f777778,
    0x3f800000,
]
EDGES = [float(np.uint32(b).view(np.float32)) for b in EDGE_BITS]
NE = len(EDGES)   # 31

_CACHE = {}


def build_bass():
    """Build + compile the per-core Bass/Tile kernel (SPMD, same NEFF on
    all 8 cores)."""
    from contextlib import ExitStack

    import concourse.bacc as bacc
    import concourse.tile as tile
    from concourse import bass, mybir

    f32 = mybir.dt.float32
    f32r = mybir.dt.float32r
    fp16 = mybir.dt.float16
    bf16 = mybir.dt.bfloat16
    fp8 = mybir.dt.float8e4
    i32 = mybir.dt.int32
    DR = mybir.MatmulPerfMode.DoubleRow

    nc = bacc.Bacc(
        "TRN2", target_bir_lowering=False, debug=False, num_devices=NCORES
    )
    feat = nc.dram_tensor("features", [SHARD, D], f32, kind="ExternalInput").ap()
    targ = nc.dram_tensor("target", [SHARD, 2], f32, kind="ExternalInput").ap()
    cent = nc.dram_tensor("centers", [K, D], f32, kind="ExternalInput").ap()
    out = nc.dram_tensor("out", [1, 1], f32, kind="ExternalOutput").ap()

    with tile.TileContext(nc) as tc, ExitStack() as ctx:
        const_p = ctx.enter_context(tc.tile_pool(name="const", bufs=1))
        work_p = ctx.enter_context(tc.tile_pool(name="work", bufs=1))
        gat_p = ctx.enter_context(tc.tile_pool(name="gat", bufs=6))
        sel_p = ctx.enter_context(tc.tile_pool(name="sel", bufs=3))
        dif_p = ctx.enter_context(tc.tile_pool(name="dif", bufs=3))
        psum_t = ctx.enter_context(tc.tile_pool(name="psumt", bufs=2, space="PSUM"))
        psum_d = ctx.enter_context(tc.tile_pool(name="psumd", bufs=2, space="PSUM"))
        dram_p = ctx.enter_context(tc.tile_pool(name="dram", bufs=1, space="DRAM"))

        # ---- sync-queue DMAs: target, centers, then features (ordered)
        ttile = work_p.tile([P, R, 2], f32)
        nc.sync.dma_start(ttile[:], targ.rearrange("(p r) c -> p r c", p=P))

        F = work_p.tile([P, R, D], f32)
        feat_re = feat.rearrange("(p r) d -> p r d", p=P)
        # PE slots (16..31) early so the PE pipeline is fed; gather-slot
        # chunks interleave to match the SWDGE gather/subtract timeline.
        forder = [(24, 26), (26, 30), (0, 4), (30, 32), (4, 8), (8, 12),
                  (12, 16), (16, 20), (20, 24)]
        for (a, b) in forder:
            nc.sync.dma_start(F[:, a:b, :], feat_re[:, a:b, :])

        # ---- fp8 centers tiles (cast emitted in the master schedule)
        cent8_sb = work_p.tile([P, NKC, D], fp8)
        table8 = dram_p.tile([K, D], fp8)

        # ---- consts
        etile = const_p.tile([P, NE], f32)
        for j, e in enumerate(EDGES):
            nc.vector.memset(etile[:, j : j + 1], e)

        # identity for PE transposes
        identity = const_p.tile([P, P], f32)
        nc.gpsimd.memset(identity[:], 0.0)
        nc.gpsimd.affine_select(
            out=identity[:],
            in_=identity[:],
            compare_op=mybir.AluOpType.not_equal,
            fill=1.0,
            base=0,
            pattern=[[-1, P]],
            channel_multiplier=1,
        )
        # iota16[j, c] = 128*c + j (bin id of partition j in chunk c)
        iota16 = const_p.tile([P, NKC], fp16)
        nc.gpsimd.iota(
            iota16[:],
            pattern=[[P, NKC]],
            base=0,
            channel_multiplier=1,
            allow_small_or_imprecise_dtypes=True,
        )
        iota_full = const_p.tile([P, NKC, P], fp16)
        nc.vector.tensor_copy(
            out=iota_full[:], in_=iota16[:].unsqueeze(2).broadcast_to([P, NKC, P])
        )
        acc = work_p.tile([P, R], f32)
        nc.vector.memset(acc[:], 0.0)
        ones = const_p.tile([P, 1], f32)
        nc.vector.memset(ones[:], 1.0)

        # ---- binning (DVE), 2 groups: PE slots first
        labi = work_p.tile([P, R, 1], i32)
        labf2 = work_p.tile([P, R, 1], f32)
        bins = work_p.tile([P, R, 2], f32)
        labf = work_p.tile([P, R, 1], f32)
        cmp = work_p.tile([P, 2 * XG, NE], f32)

        def bin_group(h0, h1):
            rs = slice(h0, h1)
            w = 2 * (h1 - h0)
            tvals = ttile[:, rs, :].rearrange("p r c -> p (r c)")
            nc.vector.tensor_tensor(
                out=cmp[:, :w, :],
                in0=tvals.unsqueeze(2).broadcast_to([P, w, NE]),
                in1=etile[:].unsqueeze(1).broadcast_to([P, w, NE]),
                op=mybir.AluOpType.is_gt,
            )
            nc.vector.tensor_reduce(
                out=bins[:, rs, :].rearrange("p r c -> p (r c)"),
                in_=cmp[:, :w, :],
                axis=mybir.AxisListType.X,
                op=mybir.AluOpType.add,
            )
            nc.vector.tensor_scalar(
                out=labf[:, rs, :],
                in0=bins[:, rs, 0:1],
                scalar1=float(32.0),
                scalar2=None,
                op0=mybir.AluOpType.mult,
            )
            nc.vector.tensor_tensor(
                out=labf2[:, rs, :],
                in0=labf[:, rs, :],
                in1=bins[:, rs, 1:2],
                op=mybir.AluOpType.add,
            )
            nc.vector.tensor_copy(out=labi[:, rs, :], in_=labf2[:, rs, :])


        # ---- gather path (slots 0..XG-1); last groups smaller so the
        # end-of-kernel subtract/square tail is short
        GGRP = [(0, 4), (4, 8), (8, 12), (12, 16), (16, 20), (20, 22), (22, 24)]
        assert GGRP[-1][1] == XG
        g8t = {}

        def gen4(k):
            a, b = GGRP[k]
            g8 = gat_p.tile([P, 4, D], fp8, tag="g8")
            for t in range(b - a):
                nc.gpsimd.indirect_dma_start(
                    out=g8[:, t, :],
                    out_offset=None,
                    in_=table8[:, :],
                    in_offset=bass.IndirectOffsetOnAxis(
                        ap=labi[:, a + t, :], axis=0
                    ),
                )
            g8t[k] = g8

        def gsub(k):
            a, b = GGRP[k]
            g8 = g8t.pop(k)
            dt_ = dif_p.tile([P, 4, D], bf16, tag="d")
            nc.vector.tensor_tensor(
                out=dt_[:, : b - a, :],
                in0=F[:, a:b, :],
                in1=g8[:, : b - a, :],
                op=mybir.AluOpType.subtract,
            )
            nc.scalar.activation(
                out=dt_[:, : b - a, :],
                in_=dt_[:, : b - a, :],
                func=mybir.ActivationFunctionType.Square,
                accum_out=acc[:, a : a + 1],
            )

        # ---- PE DoubleRow one-hot path (slots XG..31), 2-tile groups
        psT2s = {}
        sels = {}
        gp2s = {}

        def TT(G):
            psT2 = psum_t.tile([P, 2, P], f32, tag="psT")
            for h in range(2):
                s = XG + 2 * G + h
                nc.tensor.transpose(
                    out=psT2[:, h, :],
                    in_=labf2[:, s, :].to_broadcast([P, P]),
                    identity=identity[:],
                )
            psT2s[G] = psT2

        def sel2(G):
            psT2 = psT2s.pop(G)
            sel = sel_p.tile([P, 2, NKC, P], fp8, tag="sel")
            nc.vector.tensor_tensor(
                out=sel[:],
                in0=psT2[:].unsqueeze(2).broadcast_to([P, 2, NKC, P]),
                in1=iota_full[:].unsqueeze(1).broadcast_to([P, 2, NKC, P]),
                op=mybir.AluOpType.is_equal,
            )
            sels[G] = sel

        def mm2(G):
            sel = sels.pop(G)
            gp2 = psum_d.tile([P, 2, D], f32, tag="gp2")
            for h in range(2):
                for c2 in range(NKC // 2):
                    nc.tensor.matmul(
                        out=gp2[:, h, :],
                        lhsT=sel[:, h, 2 * c2 : 2 * c2 + 2, :],
                        rhs=cent8_sb[:, 2 * c2 : 2 * c2 + 2, :],
                        start=(c2 == 0),
                        stop=(c2 == NKC // 2 - 1),
                        perf_mode=DR,
                    )
            gp2s[G] = gp2

        def sq2(G):
            gp2 = gp2s.pop(G)
            s0 = XG + 2 * G
            dt_ = dif_p.tile([P, 2, D], bf16, tag="d2")
            nc.vector.tensor_tensor(
                out=dt_[:],
                in0=F[:, s0 : s0 + 2, :],
                in1=gp2[:],
                op=mybir.AluOpType.subtract,
            )
            nc.scalar.activation(
                out=dt_[:],
                in_=dt_[:],
                func=mybir.ActivationFunctionType.Square,
                accum_out=acc[:, s0 : s0 + 1],
            )

        # ---- master emission schedule
        nc.gpsimd.dma_start(out=table8[:, :], in_=cent[:, :])
        nc.gpsimd.dma_start(    # before gens: table8 write->read dep
            cent8_sb[:, :, :], table8[:].rearrange("(c p) d -> p c d", p=P)
        )
        bin_group(0, XG)        # gather slots first
        for k in range(len(GGRP)):
            gen4(k)
        bin_group(XG, R)

        TT(0)
        sel2(0)
        for G in range(NG):
            if G + 1 < NG:
                TT(G + 1)
                sel2(G + 1)
            mm2(G)
            sq2(G)
        for k in range(len(GGRP)):
            gsub(k)

        # ---- final reduction
        s_ = work_p.tile([P, 1], f32)
        nc.vector.tensor_reduce(
            out=s_[:], in_=acc[:], axis=mybir.AxisListType.X, op=mybir.AluOpType.add
        )
        ps = psum_t.tile([1, 1], f32)
        nc.tensor.matmul(out=ps[:], lhsT=ones[:], rhs=s_[:], start=True, stop=True)
        res = work_p.tile([1, 1], f32)
        nc.vector.tensor_copy(out=res[:], in_=ps[:])
        nc.sync.dma_start(out[:, :], res[:])

    nc.compile()
    return nc


def _get_nc():
    if "nc" not in _CACHE:
        _CACHE["nc"] = build_bass()
    return _CACHE["nc"]


def kernel(features, target, centers):
    from concourse.bass_utils import run_bass_kernel_spmd

    features = np.ascontiguousarray(features, dtype=np.float32)
    target = np.ascontiguousarray(target, dtype=np.float32)
    centers = np.ascontiguousarray(centers, dtype=np.float32)

    nc = _get_nc()
    in_maps = []
    for c in range(NCORES):
        sl = slice(c * SHARD, (c + 1) * SHARD)
        in_maps.append(
            {
                "features": np.ascontiguousarray(features[sl]),
                "target": np.ascontiguousarray(target[sl]),
                "centers": centers,
            }
        )
    r = run_bass_kernel_spmd(
        nc,
        in_maps,
        core_ids=list(range(NCORES)),
        trace=_CACHE.get("trace", False),
        tmpdir=_CACHE.get("tmpdir"),
    )
    _CACHE["last_results"] = r
    total = sum(float(res["out"][0, 0]) for res in r.results)
    return np.float32(total / N)
